# revision 21
# baseline (speedup 1.0000x reference)
"""MoE-ALU (add with carry + xor over one-hot byte encodings) on 8 NeuronCores.

Semantics (validated against the jax reference bit-exactly): inputs a, b are
exact one-hot byte encodings [B, 4, 256] (little-endian bytes of 32-bit ints);
with SCALE=100 every softmax in the reference collapses to an exact one-hot, so

    out[0] = one_hot bytes of (a_int + b_int) mod 2^32
    out[1] = one_hot bytes of (a_int ^ b_int)

Device kernel (pure data parallel, batch sharded over 8 cores), raw Bass
(this toolchain's walrus encodes at most ONE sync wait per instruction, so
Tile-generated schedules don't compile; manual sems with standalone waits do):

  per 128-row tile (a|b side by side in one [128, 2048] SBUF tile):
    decode  4x scalar_tensor_tensor with accum_out: multiply one 512-col
            segment by the [0..255 | 0,256,...,65280] pattern and reduce in
            one op -> a_lo a_hi b_lo b_hi (16-bit halves, f32-exact)
    add     int halves add; carry folded in via one (s_lo>=65536)+s_hi STT
    xor     int32 xor of the halves
    bytes   int32 shift/mask -> 8 byte indices
    encode  single is_equal [128, 8, 256] of the int iota table against the
            stride-0-broadcast indices, writing f32 one-hots directly

  engines: SyncE issues input DMAs, ScalarE issues output DMAs, VectorE
  computes. Rotating per-buffer-slot semaphores make DMA-queue completion
  order irrelevant (slot reuse is gated by the compute semaphore).

  DVE ops overlap in the engine pipe and do NOT self-interlock (measured:
  removing sync gives stale reads), so every same-engine RAW step waits on a
  monotonically counted DVE semaphore; per-tile temporaries are
  parity-double-buffered so consecutive tiles can overlap, with cross-parity
  reuse gated by the compute semaphore of tile i-1.
"""
from contextlib import ExitStack

import numpy as np

import concourse.bass as bass
from concourse import mybir
from concourse.bass_utils import run_bass_kernel_spmd

F32 = mybir.dt.float32
I32 = mybir.dt.int32

P = 128
N_CORES = 8
B = 32768
B_LOC = B // N_CORES          # 4096 rows per core
ROW = 4 * 256                 # 1024 f32 per row per tensor
N_TILES = B_LOC // P          # 32

NBUF = 8                      # input buffer slots
OBUF = 7                      # output buffer slots

TABI_COLS = 2048 + 8          # encode iota x8 | shift pattern


def _build_nc() -> bass.Bass:
    nc = bass.Bass(trn_type="TRN2")
    a_d = nc.dram_tensor("a", [B_LOC, ROW], F32, kind="ExternalInput")
    b_d = nc.dram_tensor("b", [B_LOC, ROW], F32, kind="ExternalInput")
    tabf_d = nc.dram_tensor("tabf", [P, 512], F32, kind="ExternalInput")
    tabi_d = nc.dram_tensor("tabi", [P, TABI_COLS], I32, kind="ExternalInput")
    out_d = nc.dram_tensor("out", [2, B_LOC, ROW], F32, kind="ExternalOutput")

    with ExitStack() as ctx:
        sb = lambda name, shape, dt: ctx.enter_context(
            nc.sbuf_tensor(name, shape, dt))
        tabf_t = sb("tabf_t", [P, 512], F32)
        tabi_t = sb("tabi_t", [P, TABI_COLS], I32)
        ab_t = [sb(f"ab_t{k}", [P, 2 * ROW], F32) for k in range(NBUF)]
        out_t = [sb(f"out_t{k}", [P, 2 * ROW], F32) for k in range(OBUF)]
        dump = [[sb(f"dump{p}_{k}", [P, 512], F32) for k in range(4)]
                for p in range(2)]
        # parity-double-buffered per-tile temporaries
        t6 = [sb(f"t6_{p}", [P, 6], F32) for p in range(2)]
        iv = [sb(f"iv_{p}", [P, 6], I32) for p in range(2)]   # a16 b16 s16
        v4 = [sb(f"v4_{p}", [P, 4], I32) for p in range(2)]   # slo shi xlo xhi
        sh8 = [sb(f"sh8_{p}", [P, 8], I32) for p in range(2)]
        idx8 = [sb(f"idx8_{p}", [P, 8], I32) for p in range(2)]

        dec = tabf_t[:]
        enc = tabi_t[:, 0:2048].rearrange("p (e k) -> p e k", k=256)
        shifts = tabi_t[:, 2048:2056].rearrange("p (a two) -> p a two", two=2)

        s_tab = ctx.enter_context(nc.semaphore("s_tab"))
        s_tab2 = ctx.enter_context(nc.semaphore("s_tab2"))
        s_load = [ctx.enter_context(nc.semaphore(f"s_load{j}"))
                  for j in range(NBUF)]
        s_store = [ctx.enter_context(nc.semaphore(f"s_store{j}"))
                   for j in range(OBUF)]
        s_comp = ctx.enter_context(nc.semaphore("s_comp"))
        s_dve = ctx.enter_context(nc.semaphore("s_dve"))

        block = ctx.enter_context(nc.Block())

        @block.sync
        def _(sync: bass.BassEngine):
            sync.dma_start(out=tabf_t[:], in_=tabf_d[:]).then_inc(s_tab, 16)
            sync.dma_start(out=tabi_t[:], in_=tabi_d[:]).then_inc(s_tab2, 16)
            for i in range(N_TILES):
                j = i % NBUF
                if i >= NBUF:
                    # slot reuse: tile i-NBUF must be fully consumed
                    sync.wait_ge(s_comp, 2 * (i - NBUF + 1))
                r0 = i * P
                sync.dma_start(
                    out=ab_t[j][:, 0:ROW], in_=a_d[r0:r0 + P, :]
                ).then_inc(s_load[j], 16)
                sync.dma_start(
                    out=ab_t[j][:, ROW:2 * ROW], in_=b_d[r0:r0 + P, :]
                ).then_inc(s_load[j], 16)

        @block.scalar
        def _(scalar: bass.BassEngine):
            for i in range(N_TILES):
                j = i % OBUF
                r0 = i * P
                scalar.wait_ge(s_comp, 2 * i + 1)
                scalar.dma_start(
                    out=out_d[0, r0:r0 + P, :], in_=out_t[j][:, 0:ROW]
                ).then_inc(s_store[j], 16)
                scalar.wait_ge(s_comp, 2 * i + 2)
                scalar.dma_start(
                    out=out_d[1, r0:r0 + P, :], in_=out_t[j][:, ROW:2 * ROW]
                ).then_inc(s_store[j], 16)

        @block.vector
        def _(vector: bass.BassEngine):
            n = 0  # statically tracked s_dve count

            vector.wait_ge(s_tab, 16)   # dec table (loaded first)
            for i in range(N_TILES):
                j = i % NBUF
                jo = i % OBUF
                pr = i % 2
                if i >= 2:
                    # tile i-2 (same parity) fully retired, incl. its encode,
                    # before its temporaries are reused
                    vector.wait_ge(s_comp, 2 * (i - 1))
                vector.wait_ge(s_load[j], 32 * (i // NBUF + 1))
                if i >= OBUF:
                    vector.wait_ge(s_store[jo], 32 * (i // OBUF))

                # decode: a_lo a_hi b_lo b_hi as f32 accumulators
                for k in range(4):
                    vector.scalar_tensor_tensor(
                        out=dump[pr][k][:],
                        in0=ab_t[j][:, 512 * k:512 * k + 512],
                        scalar=1.0,
                        in1=dec,
                        op0=mybir.AluOpType.mult,
                        op1=mybir.AluOpType.mult,
                        accum_out=t6[pr][:, k:k + 1],
                    ).then_inc(s_dve, 1)
                n += 4
                vector.wait_ge(s_dve, n)
                # int cast of the four halves
                vector.tensor_copy(iv[pr][:, 0:4], t6[pr][:, 0:4]).then_inc(
                    s_dve, 1)
                n += 1
                vector.wait_ge(s_dve, n)
                # s16 halves sum and xor halves
                vector.tensor_tensor(
                    out=iv[pr][:, 4:6], in0=iv[pr][:, 0:2],
                    in1=iv[pr][:, 2:4],
                    op=mybir.AluOpType.add).then_inc(s_dve, 1)
                vector.tensor_tensor(
                    out=v4[pr][:, 2:4], in0=iv[pr][:, 0:2],
                    in1=iv[pr][:, 2:4],
                    op=mybir.AluOpType.bitwise_xor).then_inc(s_dve, 1)
                n += 2
                vector.wait_ge(s_dve, n)
                # carry lo->hi: s_lo' = s_lo & 65535 ; s_hi' = (s_lo>=2^16)+s_hi
                vector.tensor_scalar(
                    out=v4[pr][:, 0:1], in0=iv[pr][:, 4:5], scalar1=65535,
                    scalar2=None,
                    op0=mybir.AluOpType.bitwise_and).then_inc(s_dve, 1)
                vector.scalar_tensor_tensor(
                    out=v4[pr][:, 1:2], in0=iv[pr][:, 4:5], scalar=65536,
                    in1=iv[pr][:, 5:6],
                    op0=mybir.AluOpType.is_ge,
                    op1=mybir.AluOpType.add).then_inc(s_dve, 1)
                n += 2
                vector.wait_ge(s_dve, n)
                if i == 0:
                    vector.wait_ge(s_tab2, 16)  # shift/enc table ready
                vector.tensor_tensor(
                    out=sh8[pr][:],
                    in0=v4[pr][:, :, None].to_broadcast((P, 4, 2)),
                    in1=shifts,
                    op=mybir.AluOpType.logical_shift_right).then_inc(s_dve, 1)
                n += 1
                vector.wait_ge(s_dve, n)
                vector.tensor_scalar(
                    out=idx8[pr][:], in0=sh8[pr][:], scalar1=255,
                    scalar2=None,
                    op0=mybir.AluOpType.bitwise_and).then_inc(s_dve, 1)
                n += 1
                vector.wait_ge(s_dve, n)
                # encode in two halves so the add-half store releases early
                vector.tensor_tensor(
                    out=out_t[jo][:, 0:ROW].rearrange(
                        "p (e k) -> p e k", k=256),
                    in0=enc[:, 0:4, :],
                    in1=idx8[pr][:, 0:4, None].to_broadcast((P, 4, 256)),
                    op=mybir.AluOpType.is_equal,
                ).then_inc(s_comp, 1)
                vector.tensor_tensor(
                    out=out_t[jo][:, ROW:2 * ROW].rearrange(
                        "p (e k) -> p e k", k=256),
                    in0=enc[:, 4:8, :],
                    in1=idx8[pr][:, 4:8, None].to_broadcast((P, 4, 256)),
                    op=mybir.AluOpType.is_equal,
                ).then_inc(s_comp, 1)

    return nc


def _make_tables():
    dec = np.concatenate([np.arange(256), np.arange(256) * 256]).astype(np.float32)
    tabf = np.tile(dec[None, :], (P, 1))
    enc = np.tile(np.arange(256, dtype=np.int64), 8)
    shifts = np.array([0, 8] * 4, np.int64)
    tabi = np.tile(np.concatenate([enc, shifts]).astype(np.int32)[None, :],
                   (P, 1))
    return tabf, tabi


_NC_CACHE = {}


def _get_nc(variant: str = "main"):
    if variant not in _NC_CACHE:
        _NC_CACHE[variant] = _build_nc()
    return _NC_CACHE[variant]


def _run(a: np.ndarray, b: np.ndarray, **spmd_kwargs):
    assert a.shape == (B, 4, 256) and b.shape == (B, 4, 256)
    a2 = np.ascontiguousarray(a, dtype=np.float32).reshape(B, ROW)
    b2 = np.ascontiguousarray(b, dtype=np.float32).reshape(B, ROW)
    tabf, tabi = _make_tables()
    in_maps = [
        {
            "a": a2[i * B_LOC:(i + 1) * B_LOC],
            "b": b2[i * B_LOC:(i + 1) * B_LOC],
            "tabf": tabf,
            "tabi": tabi,
        }
        for i in range(N_CORES)
    ]
    nc = _get_nc()
    kr = run_bass_kernel_spmd(nc, in_maps, list(range(N_CORES)), **spmd_kwargs)
    shards = [kr.results[i]["out"] for i in range(N_CORES)]
    out = np.concatenate(shards, axis=1).reshape(2, B, 4, 256)
    return out, kr


def kernel(a: np.ndarray, b: np.ndarray) -> np.ndarray:
    out, _ = _run(a, b)
    return out


# revision 23
# speedup vs baseline: 1.0035x; 1.0035x over previous
"""MoE-ALU (add with carry + xor over one-hot byte encodings) on 8 NeuronCores.

Semantics (validated against the jax reference bit-exactly): inputs a, b are
exact one-hot byte encodings [B, 4, 256] (little-endian bytes of 32-bit ints);
with SCALE=100 every softmax in the reference collapses to an exact one-hot, so

    out[0] = one_hot bytes of (a_int + b_int) mod 2^32
    out[1] = one_hot bytes of (a_int ^ b_int)

Device kernel (pure data parallel, batch sharded over 8 cores), raw Bass
(this toolchain's walrus encodes at most ONE sync wait per instruction, so
Tile-generated schedules don't compile; manual sems with standalone waits do):

  per 128-row tile (a|b side by side in one [128, 2048] SBUF tile):
    decode  4x scalar_tensor_tensor with accum_out: multiply one 512-col
            segment by the [0..255 | 0,256,...,65280] pattern and reduce in
            one op -> a_lo a_hi b_lo b_hi (16-bit halves, f32-exact)
    add     int halves add; carry folded in via one (s_lo>=65536)+s_hi STT
    xor     int32 xor of the halves
    bytes   int32 shift/mask -> 8 byte indices
    encode  single is_equal [128, 8, 256] of the int iota table against the
            stride-0-broadcast indices, writing f32 one-hots directly

  engines: SyncE issues input DMAs, ScalarE issues output DMAs, VectorE
  computes. Rotating per-buffer-slot semaphores make DMA-queue completion
  order irrelevant (slot reuse is gated by the compute semaphore).

  DVE ops overlap in the engine pipe and do NOT self-interlock (measured:
  removing sync gives stale reads), so every same-engine RAW step waits on a
  monotonically counted DVE semaphore; per-tile temporaries are
  parity-double-buffered so consecutive tiles can overlap, with cross-parity
  reuse gated by the compute semaphore of tile i-1.
"""
from contextlib import ExitStack

import numpy as np

import concourse.bass as bass
from concourse import mybir
from concourse.bass_utils import run_bass_kernel_spmd

F32 = mybir.dt.float32
I32 = mybir.dt.int32

P = 128
N_CORES = 8
B = 32768
B_LOC = B // N_CORES          # 4096 rows per core
ROW = 4 * 256                 # 1024 f32 per row per tensor
N_TILES = B_LOC // P          # 32

NBUF = 8                      # input buffer slots
OBUF = 7                      # output buffer slots

TABI_COLS = 2048 + 8          # encode iota x8 | shift pattern


def _build_nc() -> bass.Bass:
    nc = bass.Bass(trn_type="TRN2")
    a_d = nc.dram_tensor("a", [B_LOC, ROW], F32, kind="ExternalInput")
    b_d = nc.dram_tensor("b", [B_LOC, ROW], F32, kind="ExternalInput")
    tabf_d = nc.dram_tensor("tabf", [P, 512], F32, kind="ExternalInput")
    tabi_d = nc.dram_tensor("tabi", [P, TABI_COLS], I32, kind="ExternalInput")
    out_d = nc.dram_tensor("out", [2, B_LOC, ROW], F32, kind="ExternalOutput")

    with ExitStack() as ctx:
        sb = lambda name, shape, dt: ctx.enter_context(
            nc.sbuf_tensor(name, shape, dt))
        tabf_t = sb("tabf_t", [P, 512], F32)
        tabi_t = sb("tabi_t", [P, TABI_COLS], I32)
        ab_t = [sb(f"ab_t{k}", [P, 2 * ROW], F32) for k in range(NBUF)]
        out_t = [sb(f"out_t{k}", [P, 2 * ROW], F32) for k in range(OBUF)]
        dump = [[sb(f"dump{p}_{k}", [P, 512], F32) for k in range(4)]
                for p in range(2)]
        # parity-double-buffered per-tile temporaries
        t6 = [sb(f"t6_{p}", [P, 6], F32) for p in range(2)]
        iv = [sb(f"iv_{p}", [P, 6], I32) for p in range(2)]   # a16 b16 s16
        v4 = [sb(f"v4_{p}", [P, 4], I32) for p in range(2)]   # slo shi xlo xhi
        sh8 = [sb(f"sh8_{p}", [P, 8], I32) for p in range(2)]
        idx8 = [sb(f"idx8_{p}", [P, 8], I32) for p in range(2)]

        dec = tabf_t[:]
        enc = tabi_t[:, 0:2048].rearrange("p (e k) -> p e k", k=256)
        shifts = tabi_t[:, 2048:2056].rearrange("p (a two) -> p a two", two=2)

        s_tab = ctx.enter_context(nc.semaphore("s_tab"))
        s_tab2 = ctx.enter_context(nc.semaphore("s_tab2"))
        s_load = [ctx.enter_context(nc.semaphore(f"s_load{j}"))
                  for j in range(NBUF)]
        s_store = [ctx.enter_context(nc.semaphore(f"s_store{j}"))
                   for j in range(OBUF)]
        s_comp = ctx.enter_context(nc.semaphore("s_comp"))
        s_dve = ctx.enter_context(nc.semaphore("s_dve"))

        block = ctx.enter_context(nc.Block())

        @block.sync
        def _(sync: bass.BassEngine):
            sync.dma_start(out=tabf_t[:], in_=tabf_d[:]).then_inc(s_tab, 16)
            for i in range(N_TILES):
                j = i % NBUF
                if i >= NBUF:
                    # slot reuse: tile i-NBUF must be fully consumed
                    sync.wait_ge(s_comp, 2 * (i - NBUF + 1))
                r0 = i * P
                sync.dma_start(
                    out=ab_t[j][:, 0:ROW], in_=a_d[r0:r0 + P, :]
                ).then_inc(s_load[j], 16)
                sync.dma_start(
                    out=ab_t[j][:, ROW:2 * ROW], in_=b_d[r0:r0 + P, :]
                ).then_inc(s_load[j], 16)
                if i == 0:
                    # big enc/shift table: after tile-0 data so compute
                    # starts sooner; needed only ~3us into tile 0
                    sync.dma_start(
                        out=tabi_t[:], in_=tabi_d[:]).then_inc(s_tab2, 16)

        @block.scalar
        def _(scalar: bass.BassEngine):
            for i in range(N_TILES):
                j = i % OBUF
                r0 = i * P
                scalar.wait_ge(s_comp, 2 * i + 1)
                scalar.dma_start(
                    out=out_d[0, r0:r0 + P, :], in_=out_t[j][:, 0:ROW]
                ).then_inc(s_store[j], 16)
                scalar.wait_ge(s_comp, 2 * i + 2)
                scalar.dma_start(
                    out=out_d[1, r0:r0 + P, :], in_=out_t[j][:, ROW:2 * ROW]
                ).then_inc(s_store[j], 16)

        @block.vector
        def _(vector: bass.BassEngine):
            n = 0  # statically tracked s_dve count

            vector.wait_ge(s_tab, 16)   # dec table (loaded first)
            for i in range(N_TILES):
                j = i % NBUF
                jo = i % OBUF
                pr = i % 2
                if i >= 2:
                    # tile i-2 (same parity) fully retired, incl. its encode,
                    # before its temporaries are reused
                    vector.wait_ge(s_comp, 2 * (i - 1))
                vector.wait_ge(s_load[j], 32 * (i // NBUF + 1))
                if i >= OBUF:
                    vector.wait_ge(s_store[jo], 32 * (i // OBUF))

                # decode: a_lo a_hi b_lo b_hi as f32 accumulators
                for k in range(4):
                    vector.scalar_tensor_tensor(
                        out=dump[pr][k][:],
                        in0=ab_t[j][:, 512 * k:512 * k + 512],
                        scalar=1.0,
                        in1=dec,
                        op0=mybir.AluOpType.mult,
                        op1=mybir.AluOpType.mult,
                        accum_out=t6[pr][:, k:k + 1],
                    ).then_inc(s_dve, 1)
                n += 4
                vector.wait_ge(s_dve, n)
                # int cast of the four halves
                vector.tensor_copy(iv[pr][:, 0:4], t6[pr][:, 0:4]).then_inc(
                    s_dve, 1)
                n += 1
                vector.wait_ge(s_dve, n)
                # s16 halves sum and xor halves
                vector.tensor_tensor(
                    out=iv[pr][:, 4:6], in0=iv[pr][:, 0:2],
                    in1=iv[pr][:, 2:4],
                    op=mybir.AluOpType.add).then_inc(s_dve, 1)
                vector.tensor_tensor(
                    out=v4[pr][:, 2:4], in0=iv[pr][:, 0:2],
                    in1=iv[pr][:, 2:4],
                    op=mybir.AluOpType.bitwise_xor).then_inc(s_dve, 1)
                n += 2
                vector.wait_ge(s_dve, n)
                # carry lo->hi: s_lo' = s_lo & 65535 ; s_hi' = (s_lo>=2^16)+s_hi
                vector.tensor_scalar(
                    out=v4[pr][:, 0:1], in0=iv[pr][:, 4:5], scalar1=65535,
                    scalar2=None,
                    op0=mybir.AluOpType.bitwise_and).then_inc(s_dve, 1)
                vector.scalar_tensor_tensor(
                    out=v4[pr][:, 1:2], in0=iv[pr][:, 4:5], scalar=65536,
                    in1=iv[pr][:, 5:6],
                    op0=mybir.AluOpType.is_ge,
                    op1=mybir.AluOpType.add).then_inc(s_dve, 1)
                n += 2
                vector.wait_ge(s_dve, n)
                if i == 0:
                    vector.wait_ge(s_tab2, 16)  # shift/enc table ready
                vector.tensor_tensor(
                    out=sh8[pr][:],
                    in0=v4[pr][:, :, None].to_broadcast((P, 4, 2)),
                    in1=shifts,
                    op=mybir.AluOpType.logical_shift_right).then_inc(s_dve, 1)
                n += 1
                vector.wait_ge(s_dve, n)
                vector.tensor_scalar(
                    out=idx8[pr][:], in0=sh8[pr][:], scalar1=255,
                    scalar2=None,
                    op0=mybir.AluOpType.bitwise_and).then_inc(s_dve, 1)
                n += 1
                vector.wait_ge(s_dve, n)
                # encode in two halves so the add-half store releases early
                vector.tensor_tensor(
                    out=out_t[jo][:, 0:ROW].rearrange(
                        "p (e k) -> p e k", k=256),
                    in0=enc[:, 0:4, :],
                    in1=idx8[pr][:, 0:4, None].to_broadcast((P, 4, 256)),
                    op=mybir.AluOpType.is_equal,
                ).then_inc(s_comp, 1)
                vector.tensor_tensor(
                    out=out_t[jo][:, ROW:2 * ROW].rearrange(
                        "p (e k) -> p e k", k=256),
                    in0=enc[:, 4:8, :],
                    in1=idx8[pr][:, 4:8, None].to_broadcast((P, 4, 256)),
                    op=mybir.AluOpType.is_equal,
                ).then_inc(s_comp, 1)

    return nc


def _make_tables():
    dec = np.concatenate([np.arange(256), np.arange(256) * 256]).astype(np.float32)
    tabf = np.tile(dec[None, :], (P, 1))
    enc = np.tile(np.arange(256, dtype=np.int64), 8)
    shifts = np.array([0, 8] * 4, np.int64)
    tabi = np.tile(np.concatenate([enc, shifts]).astype(np.int32)[None, :],
                   (P, 1))
    return tabf, tabi


_NC_CACHE = {}


def _get_nc(variant: str = "main"):
    if variant not in _NC_CACHE:
        _NC_CACHE[variant] = _build_nc()
    return _NC_CACHE[variant]


def _run(a: np.ndarray, b: np.ndarray, **spmd_kwargs):
    assert a.shape == (B, 4, 256) and b.shape == (B, 4, 256)
    a2 = np.ascontiguousarray(a, dtype=np.float32).reshape(B, ROW)
    b2 = np.ascontiguousarray(b, dtype=np.float32).reshape(B, ROW)
    tabf, tabi = _make_tables()
    in_maps = [
        {
            "a": a2[i * B_LOC:(i + 1) * B_LOC],
            "b": b2[i * B_LOC:(i + 1) * B_LOC],
            "tabf": tabf,
            "tabi": tabi,
        }
        for i in range(N_CORES)
    ]
    nc = _get_nc()
    kr = run_bass_kernel_spmd(nc, in_maps, list(range(N_CORES)), **spmd_kwargs)
    shards = [kr.results[i]["out"] for i in range(N_CORES)]
    out = np.concatenate(shards, axis=1).reshape(2, B, 4, 256)
    return out, kr


def kernel(a: np.ndarray, b: np.ndarray) -> np.ndarray:
    out, _ = _run(a, b)
    return out
